# revision 1
# baseline (speedup 1.0000x reference)
"""BiPhaseScorer Trainium2 kernel (8 NeuronCores, SPMD).

Sharding: core (b, g) = batch b in {0,1} x head-group g in {0..3} (2 heads each).
Each core: projects its batch's tokens onto its 2 heads' QKV slices, runs
bi-phase attention (phase + magnitude scores, softmax), applies its heads'
slice of the output projections, and writes per-core partial outputs
[S, E] for x and y channels. Host sums partials over head-groups + bias.

Math: cos(arctan2(y,x)) = x/r, sin = y/r with r = sqrt(x^2+y^2), so
  scores = BETA*(cosq.cosk + sinq.sink)/D + (1-BETA)*(rq.rk)/sqrt(D)
is three rank-D contractions; cos/sin stack to one K=128 matmul. All matmul
scaling is folded into the Q-side host-prep (weights scaled by ALPHA) and the
on-device sqrt (scale=1/C1^2), so scores come out of PSUM fully scaled.
Softmax skips max-subtraction (scores bounded ~O(10) for this regime; exp is
safe in fp32). Denominator via ones-vector matmul; normalization applied to
O^T via a rank-1 broadcast matmul + multiply.

Layouts (per core):
  cs_q[h]/cs_k[h] [128, S] f32r: h0 = [cos(0:64); sin(64:128)], h1 = [sin; cos]
  r_q/r_k         [128, S] f32r: heads packed [h0 | h1] on partitions
  v_sb            [128, KC, 256] f32r: per key-chunk, free = per-head 128 cols,
                  h0 = [vx|vy], h1 = [vy|vx] (swap lets O^T land lane-local
                  in the x/y-stacked ot tensors below)
  otx             [128, S] f32r: [h0 Ox^T (0:64); h1 Ox^T (64:128)]
  oty             [128, S] f32r: [h1 Oy^T (0:64); h0 Oy^T (64:128)]
Out-proj is then a single Kc=128 matmul per (token block, channel).
V projections run in bf16 (inputs+weights); everything else f32r.
"""

import numpy as np
import ml_dtypes

import concourse.bacc as bacc
import concourse.mybir as mybir
from concourse.tile import TileContext
from concourse.bass_utils import run_bass_kernel_spmd

B, S, E, H = 2, 2048, 512, 8
D = E // H              # 64
BETA = 0.5
SCALE = float(np.sqrt(D))
C1 = BETA / D
C2 = (1.0 - BETA) / SCALE
ALPHA = C1 * C2
SQRT_SCALE = (C2 / ALPHA) ** 2  # == 1/C1^2

NCORES = 8
HG = 2
D2 = HG * D             # 128
EC = E // 128           # 4
TT = 4                  # 512-token tiles
QT = 4
KC = S // 128           # 16
TB = S // 128           # 16

F32 = mybir.dt.float32
F32R = mybir.dt.float32r
BF16 = mybir.dt.bfloat16

TRACE = False
LAST_RESULTS = None


def build_bass(stage="full"):
    nc = bacc.Bacc("TRN2", target_bir_lowering=False, debug=False,
                   enable_asserts=True, num_devices=NCORES)

    xs = {}
    for n in ["xqx", "xqy", "xkx", "xky"]:
        xs[n] = nc.dram_tensor(n, [E, S], BF16, kind="ExternalInput")
    for n in ["xvx", "xvy"]:
        xs[n] = nc.dram_tensor(n, [E, S], BF16, kind="ExternalInput")
    ws = {}
    for n in ["wqx", "wqy", "wkx", "wky"]:
        ws[n] = nc.dram_tensor(n, [E, D2], BF16, kind="ExternalInput")
    for n in ["wvx", "wvy"]:
        ws[n] = nc.dram_tensor(n, [E, D2], BF16, kind="ExternalInput")
    # x/y-stacked output projections [128, E] (see module docstring)
    wox = nc.dram_tensor("wox", [D2, E], F32R, kind="ExternalInput")
    woy = nc.dram_tensor("woy", [D2, E], F32R, kind="ExternalInput")
    bs = {n: nc.dram_tensor(n, [D2], F32, kind="ExternalInput")
          for n in ["bqx", "bqy", "bkx", "bky"]}
    yx = nc.dram_tensor("yx", [S, E], F32, kind="ExternalOutput")
    yy = nc.dram_tensor("yy", [S, E], F32, kind="ExternalOutput")
    dbg = {}
    if stage == "A":
        for n in ["d_csq0", "d_csq1", "d_csk0", "d_rq", "d_rk"]:
            dbg[n] = nc.dram_tensor(n, [128, S], F32, kind="ExternalOutput")
        dbg["d_v"] = nc.dram_tensor("d_v", [128, KC, 2 * D2], F32, kind="ExternalOutput")
    if stage == "B":
        for n in ["d_otx", "d_oty"]:
            dbg[n] = nc.dram_tensor(n, [128, S], F32, kind="ExternalOutput")

    with TileContext(nc) as tc:
        with (
            tc.tile_pool(name="persist", bufs=1) as pp,
            tc.tile_pool(name="wpool", bufs=1) as wp,
            tc.tile_pool(name="stream", bufs=4) as sp,
            tc.tile_pool(name="tmp", bufs=2) as tp,
            tc.tile_pool(name="psA", bufs=1, space="PSUM") as psA,
            tc.tile_pool(name="psB", bufs=1, space="PSUM") as psB,
            tc.tile_pool(name="psC", bufs=1, space="PSUM") as psC,
        ):
            cs_q = [[pp.tile([128, 512], F32R, tag=f"cs_q{h}_{t}", name=f"cs_q{h}_{t}")
                     for t in range(QT)] for h in range(HG)]
            cs_k = [[pp.tile([128, 512], F32R, tag=f"cs_k{h}_{t}", name=f"cs_k{h}_{t}")
                     for t in range(TT)] for h in range(HG)]
            r_q = [pp.tile([128, 512], F32R, tag=f"r_q{t}", name=f"r_q{t}") for t in range(QT)]
            r_k = [pp.tile([128, 512], F32R, tag=f"r_k{t}", name=f"r_k{t}") for t in range(TT)]
            v_sb = [pp.tile([128, 2 * D2], F32R, tag=f"v_sb{t}", name=f"v_sb{t}")
                    for t in range(KC)]
            otx = [pp.tile([128, 512], F32R, tag=f"otx{t}", name=f"otx{t}") for t in range(QT)]
            oty = [pp.tile([128, 512], F32R, tag=f"oty{t}", name=f"oty{t}") for t in range(QT)]

            w_sb = {}
            for n in ws:
                w_sb[n] = wp.tile([128, EC, D2], BF16, tag=f"w_{n}", name=f"w_{n}")
                nc.sync.dma_start(w_sb[n][:], ws[n].ap().rearrange("(c p) d -> p c d", p=128))
            wox_sb = wp.tile([D2, E], F32R, tag="wox")
            woy_sb = wp.tile([D2, E], F32R, tag="woy")
            nc.sync.dma_start(wox_sb[:], wox.ap())
            nc.sync.dma_start(woy_sb[:], woy.ap())
            b_sb = {n: wp.tile([D2, 1], F32, tag=f"b_{n}", name=f"b_{n}") for n in bs}
            for n in bs:
                nc.sync.dma_start(b_sb[n][:], bs[n].ap().unsqueeze(1))
            ones_f32 = wp.tile([128, 1], F32, tag="ones32")
            nc.vector.memset(ones_f32[:], 1.0)
            ones_den = wp.tile([128, 1], F32R, tag="ones_den")
            nc.vector.tensor_copy(ones_den[:], ones_f32[:])
            ones_row_f32 = wp.tile([1, 128], F32, tag="onesr32")
            nc.vector.memset(ones_row_f32[:], 1.0)
            ones_row = wp.tile([1, 128], F32R, tag="ones_row")
            nc.vector.tensor_copy(ones_row[:], ones_row_f32[:])
            eps_sb = wp.tile([128, 1], F32, tag="eps")
            nc.vector.memset(eps_sb[:], 1e-20)

            def qk_side(psA, ix, iy, nwx, nwy, nbx, nby, cs_t, r_t, sc, tt):
                tsl = slice(tt * 512, (tt + 1) * 512)
                r_dst = r_t[tt][:, :]
                cs0 = cs_t[0][tt]
                cs1 = cs_t[1][tt]
                csl = slice(0, 512)
                xa = sp.tile([128, EC, 512], BF16, tag="xtile", name="xa")
                nc.sync.dma_start(xa[:], xs[ix].ap().rearrange(
                    "(c p) t -> p c t", p=128)[:, :, tsl])
                xb = sp.tile([128, EC, 512], BF16, tag="xtile", name="xb")
                nc.sync.dma_start(xb[:], xs[iy].ap().rearrange(
                    "(c p) t -> p c t", p=128)[:, :, tsl])
                pa = psA.tile([128, 512], F32, tag="proj", bufs=2, name="pa")
                pb = psA.tile([128, 512], F32, tag="proj", bufs=2, name="pb")
                for ec in range(EC):
                    nc.tensor.matmul(pa[:], w_sb[nwx][:, ec], xa[:, ec],
                                     start=(ec == 0), stop=(ec == EC - 1))
                for ec in range(EC):
                    nc.tensor.matmul(pb[:], w_sb[nwy][:, ec], xb[:, ec],
                                     start=(ec == 0), stop=(ec == EC - 1))
                pxb = tp.tile([128, 512], F32, tag="pxb", name="pxb")
                pyb = tp.tile([128, 512], F32, tag="pyb", name="pyb")
                nc.vector.tensor_scalar_add(pxb[:], pa[:], b_sb[nbx][:])
                nc.vector.tensor_scalar_add(pyb[:], pb[:], b_sb[nby][:])
                t0 = tp.tile([128, 512], F32, tag="t0", name="t0")
                t1 = tp.tile([128, 512], F32, tag="t1", name="t1")
                nc.scalar.activation(t0[:], pa[:], mybir.ActivationFunctionType.Square,
                                     bias=b_sb[nbx][:])
                nc.scalar.activation(t1[:], pb[:], mybir.ActivationFunctionType.Square,
                                     bias=b_sb[nby][:])
                nc.gpsimd.tensor_add(t0[:], t0[:], t1[:])
                nc.scalar.activation(r_dst, t0[:],
                                     mybir.ActivationFunctionType.Sqrt,
                                     bias=eps_sb[:], scale=float(sc))
                rc = tp.tile([128, 512], F32, tag="rc", name="rc")
                nc.vector.reciprocal(rc[:], r_dst)
                # cos halves straight into stacks (lane-local)
                nc.vector.tensor_mul(cs0[0:64, csl], pxb[0:64], rc[0:64])
                nc.vector.tensor_mul(cs1[64:128, csl], pxb[64:128], rc[64:128])
                # sin full -> tmp, then DMA partition-shift into stacks
                sn = tp.tile([128, 512], F32R, tag="sn", name="sn")
                nc.vector.tensor_mul(sn[:], pyb[:], rc[:])
                nc.gpsimd.dma_start(cs0[64:128, csl], sn[0:64])
                nc.gpsimd.dma_start(cs1[0:64, csl], sn[64:128])

            if True:
                # interleaved rounds: K(tt) -> V(tt) -> Q(tt), so phase B can
                # start on early kc chunks while later tiles still project
                for tt in range(TT):
                    qk_side(psA, "xkx", "xky", "wkx", "wky", "bkx", "bky",
                            cs_k, r_k, 1.0, tt)
                    tsl = slice(tt * 512, (tt + 1) * 512)
                    xvx_t = sp.tile([128, EC, 512], BF16, tag="xv", name="xvx_t")
                    nc.sync.dma_start(xvx_t[:], xs["xvx"].ap().rearrange(
                        "(c p) t -> p c t", p=128)[:, :, tsl])
                    xvy_t = sp.tile([128, EC, 512], BF16, tag="xv", name="xvy_t")
                    nc.sync.dma_start(xvy_t[:], xs["xvy"].ap().rearrange(
                        "(c p) t -> p c t", p=128)[:, :, tsl])
                    for sub in range(4):
                        tb = tt * 4 + sub
                        ssl = slice(sub * 128, (sub + 1) * 128)
                        pv = psA.tile([128, 512], F32, tag="proj", bufs=2, name="pv")[:, 0:2 * D2]
                        for ec in range(EC):
                            nc.tensor.matmul(pv[:, 0:D2], xvx_t[:, ec, ssl],
                                             w_sb["wvx"][:, ec],
                                             start=(ec == 0), stop=False)
                        for ec in range(EC):
                            nc.tensor.matmul(pv[:, D2:2 * D2], xvy_t[:, ec, ssl],
                                             w_sb["wvy"][:, ec],
                                             start=(ec == 0), stop=(ec == EC - 1))
                        # psum input-major [vx_h0|vx_h1|vy_h0|vy_h1] ->
                        # v_sb h0 = [vx_h0|vy_h0], h1 = [vy_h1|vx_h1]
                        vt = v_sb[tb][:].rearrange("p (i z) -> p i z", i=4)
                        pvv = pv[:].rearrange("p (i z) -> p i z", i=4)
                        # out blocks (0,2) <- in blocks (0,3); out (1,3) <- in (2,1)
                        nc.vector.tensor_copy(vt[:, 0::2], pvv[:, 0::3])
                        nc.vector.tensor_copy(vt[:, 1::2], pvv[:, 2:0:-1])
                    qk_side(psA, "xqx", "xqy", "wqx", "wqy", "bqx", "bqy",
                            cs_q, r_q, SQRT_SCALE, tt)

            if stage == "A":
                for t in range(QT):
                    qsl = slice(t * 512, (t + 1) * 512)
                    nc.sync.dma_start(dbg["d_csq0"].ap()[:, qsl], cs_q[0][t][:].bitcast(F32))
                    nc.sync.dma_start(dbg["d_csq1"].ap()[:, qsl], cs_q[1][t][:].bitcast(F32))
                    nc.sync.dma_start(dbg["d_rq"].ap()[:, qsl], r_q[t][:].bitcast(F32))
                for t in range(TT):
                    qsl = slice(t * 512, (t + 1) * 512)
                    nc.sync.dma_start(dbg["d_csk0"].ap()[:, qsl], cs_k[0][t][:].bitcast(F32))
                    nc.sync.dma_start(dbg["d_rk"].ap()[:, qsl], r_k[t][:].bitcast(F32))
                for t in range(KC):
                    nc.sync.dma_start(dbg["d_v"].ap()[:, t], v_sb[t][:].bitcast(F32))

            if True:
              if stage != "A":
                for qt in range(QT):
                    for h in range(HG):
                        hsl = slice(h * 64, (h + 1) * 64)
                        po = psB.tile([128, 512], F32, tag="o", bufs=2, name="po")
                        acc_d = tp.tile([128, 512], F32R, tag="accd", bufs=2, name="acc_d")
                        acc_p = tp.tile([128, 512], F32R, tag="accp", bufs=2, name="acc_p")
                        # software-pipelined: attnV(kc) is emitted after
                        # cs/mag(kc+1) so the strict-FIFO PE never waits on
                        # ACT's exp of the current chunk
                        prev_es = None
                        for kc in range(KC):
                            kt, ko = kc // 4, (kc % 4) * 128
                            ps = psB.tile([128, 512], F32, tag="s", bufs=3, name="ps")
                            nc.tensor.matmul(ps[:], cs_k[h][kt][:, ko:ko + 128],
                                             cs_q[h][qt][:, :],
                                             start=True, stop=False)
                            nc.tensor.matmul(ps[:], r_k[kt][hsl, ko:ko + 128],
                                             r_q[qt][hsl, :],
                                             start=False, stop=True)
                            es = tp.tile([128, 512], F32R, tag="es", bufs=4, name="es")
                            nc.scalar.activation(es[:], ps[:],
                                                 mybir.ActivationFunctionType.Exp)
                            if prev_es is not None:
                                pv_ = v_sb[kc - 1][:, h * D2:(h + 1) * D2]
                                nc.tensor.matmul(po[:], pv_, prev_es[:],
                                                 start=(kc == 1), stop=False)
                            # two serial accumulators: even chunks on DVE, odd on Pool
                            eng, acc = (nc.vector, acc_d) if kc % 2 == 0 else (nc.gpsimd, acc_p)
                            if kc < 2:
                                eng.tensor_copy(acc[:], es[:])
                            else:
                                eng.tensor_add(acc[:], acc[:], es[:])
                            prev_es = es
                        nc.tensor.matmul(po[:], v_sb[KC - 1][:, h * D2:(h + 1) * D2],
                                         prev_es[:], start=False, stop=True)
                        nc.vector.tensor_add(acc_d[:], acc_d[:], acc_p[:])
                        pden = psB.tile([1, 512], F32, tag="den", bufs=1, name="pden")
                        nc.tensor.matmul(pden[:], ones_den[:], acc_d[:],
                                         start=True, stop=True)
                        rden = tp.tile([1, 512], F32R, tag="rden", name="rden")
                        with nc.allow_low_precision(reason="f32r rounding for PE broadcast"):
                            nc.vector.reciprocal(rden[:], pden[:])
                        pbc = psB.tile([128, 512], F32, tag="s", bufs=3, name="pbc")
                        nc.tensor.matmul(pbc[:], ones_row[:], rden[:],
                                         start=True, stop=True)
                        bc_sb = tp.tile([128, 512], F32, tag="bcs", name="bc_sb")
                        nc.vector.tensor_copy(bc_sb[:], pbc[:])
                        # h0: po = [Ox^T; Oy^T] -> otx[0:64], oty[64:128]
                        # h1: po = [Oy^T; Ox^T] -> oty[0:64], otx[64:128]
                        lo, hi = (otx[qt], oty[qt]) if h == 0 else (oty[qt], otx[qt])
                        nc.vector.tensor_mul(lo[0:64, :], po[0:64], bc_sb[0:64])
                        nc.vector.tensor_mul(hi[64:128, :], po[64:128], bc_sb[64:128])

            if stage == "B":
                for t in range(QT):
                    qsl = slice(t * 512, (t + 1) * 512)
                    nc.sync.dma_start(dbg["d_otx"].ap()[:, qsl], otx[t][:].bitcast(F32))
                    nc.sync.dma_start(dbg["d_oty"].ap()[:, qsl], oty[t][:].bitcast(F32))

            if True:
              if stage == "full":
                for tb in range(TB):
                    tsl = slice(tb * 128, (tb + 1) * 128)
                    qt, sb_ = tb // 4, (tb % 4) * 128
                    pyx = psA.tile([128, E], F32, tag="proj", bufs=2, name="pyx")
                    pyy = psA.tile([128, E], F32, tag="proj", bufs=2, name="pyy")
                    nc.tensor.matmul(pyx[:], otx[qt][:, sb_:sb_ + 128], wox_sb[:],
                                     start=True, stop=True)
                    nc.tensor.matmul(pyy[:], oty[qt][:, sb_:sb_ + 128], woy_sb[:],
                                     start=True, stop=True)
                    ox = tp.tile([128, E], F32, tag="ox", name="ox")
                    oy = tp.tile([128, E], F32, tag="oy", name="oy")
                    nc.vector.tensor_copy(ox[:], pyx[:])
                    nc.vector.tensor_copy(oy[:], pyy[:])
                    nc.sync.dma_start(yx.ap()[tsl, :], ox[:])
                    nc.sync.dma_start(yy.ap()[tsl, :], oy[:])

    nc.finalize()
    return nc


_NC_CACHE = None


def make_in_maps(acts, W, bias):
    """acts: dict qx..vy [B,S,E] f32; W: dict Wqx..Woy; bias: dict bqx..boy."""
    f32 = np.float32
    bf16 = ml_dtypes.bfloat16
    in_maps = []
    for core in range(NCORES):
        b, g = core // 4, core % 4
        gs = slice(g * D2, (g + 1) * D2)
        h0 = slice((2 * g) * D, (2 * g + 1) * D)
        h1 = slice((2 * g + 1) * D, (2 * g + 2) * D)
        m = {}
        m["xqx"] = np.ascontiguousarray(acts["qx"][b].T).astype(bf16)
        m["xqy"] = np.ascontiguousarray(acts["qy"][b].T).astype(bf16)
        m["xkx"] = np.ascontiguousarray(acts["kx"][b].T).astype(bf16)
        m["xky"] = np.ascontiguousarray(acts["ky"][b].T).astype(bf16)
        m["xvx"] = np.ascontiguousarray(acts["vx"][b].T).astype(bf16)
        m["xvy"] = np.ascontiguousarray(acts["vy"][b].T).astype(bf16)
        m["wqx"] = (np.ascontiguousarray(W["Wqx"][gs].T) * f32(ALPHA)).astype(bf16)
        m["wqy"] = (np.ascontiguousarray(W["Wqy"][gs].T) * f32(ALPHA)).astype(bf16)
        m["wkx"] = np.ascontiguousarray(W["Wkx"][gs].T).astype(bf16)
        m["wky"] = np.ascontiguousarray(W["Wky"][gs].T).astype(bf16)
        m["wvx"] = np.ascontiguousarray(W["Wvx"][gs].T).astype(bf16)
        m["wvy"] = np.ascontiguousarray(W["Wvy"][gs].T).astype(bf16)
        # otx partitions = (h0 dx, h1 dx); oty partitions = (h1 dy, h0 dy)
        m["wox"] = np.ascontiguousarray(
            np.concatenate([W["Wox"][:, h0].T, W["Wox"][:, h1].T], axis=0))
        m["woy"] = np.ascontiguousarray(
            np.concatenate([W["Woy"][:, h1].T, W["Woy"][:, h0].T], axis=0))
        m["bqx"] = bias["bqx"][gs] * f32(ALPHA)
        m["bqy"] = bias["bqy"][gs] * f32(ALPHA)
        m["bkx"] = np.ascontiguousarray(bias["bkx"][gs])
        m["bky"] = np.ascontiguousarray(bias["bky"][gs])
        in_maps.append(m)
    return in_maps


def kernel(qx, qy, kx, ky, vx, vy,
           Wqx, bqx, Wqy, bqy, Wkx, bkx, Wky, bky,
           Wvx, bvx, Wvy, bvy, Wox, box, Woy, boy):
    global _NC_CACHE, LAST_RESULTS
    f32 = np.float32
    acts = {"qx": qx, "qy": qy, "kx": kx, "ky": ky, "vx": vx, "vy": vy}
    acts = {k: np.asarray(v, f32) for k, v in acts.items()}
    W = {"Wqx": Wqx, "Wqy": Wqy, "Wkx": Wkx, "Wky": Wky,
         "Wvx": Wvx, "Wvy": Wvy, "Wox": Wox, "Woy": Woy}
    W = {k: np.asarray(v, f32) for k, v in W.items()}
    bias = {"bqx": bqx, "bqy": bqy, "bkx": bkx, "bky": bky,
            "bvx": bvx, "bvy": bvy}
    bias = {k: np.asarray(v, f32) for k, v in bias.items()}
    box, boy = np.asarray(box, f32), np.asarray(boy, f32)

    if _NC_CACHE is None:
        _NC_CACHE = build_bass()
    nc = _NC_CACHE

    in_maps = make_in_maps(acts, W, bias)
    # device execution can flake (NRT_EXEC_UNIT_UNRECOVERABLE observed once
    # on an otherwise-identical program) -> retry once before giving up
    try:
        res = run_bass_kernel_spmd(nc, in_maps, core_ids=list(range(NCORES)),
                                   trace=TRACE)
    except Exception:
        import time
        time.sleep(5)
        res = run_bass_kernel_spmd(nc, in_maps, core_ids=list(range(NCORES)),
                                   trace=TRACE)
    LAST_RESULTS = res

    out_x = np.zeros((B, S, E), f32)
    out_y = np.zeros((B, S, E), f32)
    for core in range(NCORES):
        b = core // 4
        out_x[b] += res.results[core]["yx"]
        out_y[b] += res.results[core]["yy"]
    out_x += box + bias["bvx"] @ W["Wox"].T
    out_y += boy + bias["bvy"] @ W["Woy"].T
    return out_x, out_y

